# revision 18
# baseline (speedup 1.0000x reference)
"""Trainium2 Bass kernel: 1024-point FFT of real rows -> (real, imag).

Math: out = FFT_1024(x[b, :]) per row. Two folding levels over the real
input x (U[n] = x[n]+x[1024-n], V[n] = x[n]-x[1024-n]) give four real
arrays Aue/Auo/Avo/Ave of length 256 per row whose cos/sin transforms
are the four spectrum quadrants (even/odd k x real/imag):
  Xr[2m]   = Aue @ cos(2pi n m/512)   + U[256](-1)^m
  Xi[2m]   = Avo @ (-sin(2pi n m/512))
  Xr[2m+1] = Auo @ cos(pi n(2m+1)/512)
  Xi[2m+1] = Ave @ (-sin(pi n(2m+1)/512)) - V[256](-1)^m
A third radix-2 level halves the device matmul work: each quadrant
matrix C satisfies C[n, sigma(c)] = +/-(-1)^n C[n, c] for a column
involution sigma (sigma(c) = 254-c for even-k quadrants, 255-c for
odd-k), so splitting the data by row parity gives two [128x128]
products E = A[0::2] @ C[0::2, :128], O = A[1::2] @ C[1::2, :128]
and the host reconstructs both column halves with one butterfly:
quad[c] = E+O, quad[sigma(c)] = +/-(E-O). Edge terms (the U[256]
rank-1 term, k=0, k=512) are host-side rank-1 corrections; the
V[256](-1)^m term rides row 0 of the OI even matrix, which is
naturally zero. PE work per row: 8 x 128x128 MACs — half the naive
quadrant cost — so the tensor engine never paces the kernel and no
clock-ramp warmup is needed.

All device I/O is fp16 (the kernel is HBM-bandwidth-bound; fp16 of
folded data + coefficients keeps L2 relative error ~4e-4). Data ships
as [p, g, j, b] per-(group, array) 512KB chunks (row n = 2p+j is
already the parity split), group 0 first so its compute and output
drains overlap group 1's input stream. The 8 quarter-size matrices
ship packed [128, 8, 128] (2KB/partition, one 256KB DMA, first in the
stream). Per group the quadrants run in input-arrival order (ER, EI,
OR, OI); each sub-product is a single K=128 matmul per 512-col PSUM
half (PSUM: E/O tags x 2 bufs), E copies convert on vector and O
copies on scalar, and each quadrant's (E, O) slot pair is adjacent
DRAM rows drained on the gpsimd queue as soon as both copies land
(the final quadrant drains per-slot to smooth the DMA tail). Inputs
ride the sync queue. ~8.7MB/core total HBM traffic.

The host performs the pure data-expansion assembly: butterflies,
parity interleave, conjugate mirror, k=0/512 columns, rank-1 edge
corrections, final transpose, fp16->fp32 upcast. Pure data-parallel
across 8 cores, no collectives.
"""

import os
import numpy as np

N_FFT = 1024
BATCH = 16384
N_CORES = 8
B_CORE = BATCH // N_CORES  # 2048
P = 128
HALF = 512
QU = 256
GC = 1024                  # batch rows per group

_BUILD_CACHE = {}


def _constants():
    """8 packed [K=128, M=128] fp16 matrices: (E, O) per quadrant.

    packed[p, 2q+part, c] = Cq[2p+part, c] for c in [0, 128); row 0 of
    the OI even matrix (naturally zero) carries the -V[256](-1)^m term
    as the pattern (-1)^(c+1).
    """
    n = np.arange(QU, dtype=np.float64)[:, None]
    c = np.arange(QU, dtype=np.float64)[None, :]
    cer = np.cos(2 * np.pi * n * (c + 1) / 512)
    cei = -np.sin(2 * np.pi * n * (c + 1) / 512)
    cor = np.cos(np.pi * n * (2 * c + 1) / 512)
    coi = -np.sin(np.pi * n * (2 * c + 1) / 512)
    coi_e = coi[0::2, :P].copy()
    coi_e[0, :] = (-1.0) ** (np.arange(P) + 1)   # V[256] edge pattern
    mats = [
        cer[0::2, :P], cer[1::2, :P],   # ER: aue
        cei[0::2, :P], cei[1::2, :P],   # EI: avo (row 0 of E is zero)
        cor[0::2, :P], cor[1::2, :P],   # OR: auo
        coi_e, coi[1::2, :P],           # OI: ave
    ]
    packed = np.stack(mats, axis=1).astype(np.float16)  # [128, 8, 128]
    return np.ascontiguousarray(packed)


def build_nc(b_core=B_CORE):
    """Build + compile the per-core Bass program (same NEFF on all cores)."""
    import concourse.mybir as mybir
    import concourse.tile as tile
    from concourse import bacc

    f16 = mybir.dt.float16
    f32 = mybir.dt.float32

    gc = min(GC, b_core)
    n_groups = b_core // gc
    n_h = gc // HALF           # 512-col PSUM halves per group (2)

    nc = bacc.Bacc(
        "TRN2", target_bir_lowering=False, debug=False, num_devices=N_CORES
    )

    names = ("aue", "avo", "auo", "ave")   # order = DMA arrival order
    # all folded data packed in stream order: slot s = 4g + array
    data_in = nc.dram_tensor(
        "dd", [P, 4 * n_groups, 2, gc], f16, kind="ExternalInput"
    )
    coef_in = nc.dram_tensor("coef", [P, 8, P], f16, kind="ExternalInput")
    # transposed halves, group-blocked: row r = 4p + slot;
    # rt slots: [ER_E, ER_O, OR_E, OR_O]; it slots: [EI_E, EI_O, OI_E, OI_O]
    o_rt = nc.dram_tensor("o_rt", [n_groups, 2 * QU, gc], f16, kind="ExternalOutput")
    o_it = nc.dram_tensor("o_it", [n_groups, 2 * QU, gc], f16, kind="ExternalOutput")

    ort_r = o_rt.ap().rearrange("g (p t) b -> g p t b", t=4)
    oit_r = o_it.ap().rearrange("g (p t) b -> g p t b", t=4)

    with tile.TileContext(nc) as tc:
        with (
            tc.tile_pool(name="const", bufs=1) as cpool,
            tc.tile_pool(name="work", bufs=1) as wpool,
            tc.tile_pool(name="outp", bufs=2) as opool,
            tc.tile_pool(name="psm", bufs=1, space="PSUM") as psm,
        ):
            coef_sb = cpool.tile([P, 8, P], f16)
            nc.sync.dma_start(out=coef_sb[:], in_=coef_in.ap())
            # per-(group, array) 512KB chunks: group 0's arrays arrive first
            # so its compute + output drains overlap group 1's input stream.
            # The last group's final array ships as two j-half chunks so its
            # quadrant can start (and drain) half an arrival earlier.
            dat = wpool.tile([P, 4 * n_groups, 2, gc], f16, name="dat")
            # per-(group, array) 512KB chunks in stream order: each
            # quadrant's matmuls start the moment its chunk lands, so
            # output drains interleave with the input stream
            for s in range(4 * n_groups):
                nc.sync.dma_start(out=dat[:, s], in_=data_in.ap()[:, s])

            # PE-activity warmup: HBM bandwidth and engine clocks are capped
            # ~50% until a power-state promotion that follows a few us of
            # sustained tensor-engine activity (and decay ~1us after it
            # stops), so spin wide matmuls from the first possible cycle
            # until the first data chunk lands, and keep short spin bursts
            # between real matmuls later (dedicated PSUM bank, zero data
            # deps) to hold the promoted state through the write tail.
            wu_in = cpool.tile([P, 4 * P], f16)
            nc.gpsimd.memset(wu_in[:], 0.0)
            wu = psm.tile([P, HALF], f32, tag="SPIN", bufs=1)
            n_wu = 11
            for w in range(n_wu):
                nc.tensor.matmul(
                    wu[:], lhsT=wu_in[:, 0:P], rhs=wu_in[:, 0:HALF],
                    start=(w == 0), stop=(w == n_wu - 1),
                )

            def spin(n):
                for _ in range(n):
                    nc.tensor.matmul(
                        wu[:], lhsT=wu_in[:, 0:P], rhs=wu_in[:, 0:HALF],
                        start=True, stop=True,
                    )

            for g in range(n_groups):
                ortg = opool.tile([P, 4, gc], f16, tag="ortg")
                oitg = opool.tile([P, 4, gc], f16, tag="oitg")

                # quadrant-major order matches input arrival
                # (aue -> avo -> auo -> ave). Per quadrant: E and O each
                # one K=128 matmul per 512-col half; E converts on vector,
                # O on scalar so the pair converts in parallel; the (E, O)
                # slot pair is adjacent DRAM rows drained together.
                last = g == n_groups - 1
                for qi, k in enumerate(names):
                    stage, st_r = (
                        (ortg, ort_r) if qi % 2 == 0 else (oitg, oit_r)
                    )
                    base = 0 if qi < 2 else 2
                    # per-512-col-half PSUM tiles (1 bank each): the half's
                    # copy starts while the next half's matmul runs, and
                    # E/O × 2 bufs leave a free bank for the spin tile.
                    # h0 halves convert on vector, h1 on scalar.
                    for part in range(2):
                        tagp = "E" if part == 0 else "O"
                        for h in range(n_h):
                            bsl = slice(h * HALF, (h + 1) * HALF)
                            ph = psm.tile([P, HALF], f32, tag=tagp, bufs=2)
                            nc.tensor.matmul(
                                ph[:], lhsT=coef_sb[:, 2 * qi + part],
                                rhs=dat[:, 4 * g + qi, part, bsl],
                                start=True, stop=True,
                            )
                            if h == 0:
                                nc.vector.tensor_copy(
                                    out=stage[:, base + part, bsl], in_=ph[:]
                                )
                            else:
                                nc.scalar.copy(
                                    out=stage[:, base + part, bsl], in_=ph[:]
                                )
                    spin(2)
                    if last and qi >= 2:
                        # final group's last quadrants: drain each slot as
                        # soon as its copies land; the OI pair rides the
                        # (now idle) lower-latency sync queue
                        eng = nc.gpsimd if qi == 2 else nc.sync
                        eng.dma_start(
                            out=st_r[g][:, base : base + 1],
                            in_=stage[:, base : base + 1],
                        )
                        eng.dma_start(
                            out=st_r[g][:, base + 1 : base + 2],
                            in_=stage[:, base + 1 : base + 2],
                        )
                    else:
                        nc.gpsimd.dma_start(
                            out=st_r[g][:, base : base + 2],
                            in_=stage[:, base : base + 2],
                        )

            # tail spins: keep the PE (and the power state) hot while the
            # final copies convert and drain; sized to finish just before
            # the last drain lands so exec time is not extended
            spin(16)

    nc.compile()
    return nc


def _get_nc(b_core=B_CORE):
    if b_core not in _BUILD_CACHE:
        _BUILD_CACHE[b_core] = build_nc(b_core)
    return _BUILD_CACHE[b_core]


def _host_prep(x):
    """Two-level real-FFT folds (transposed) + host-side edge columns."""
    B = x.shape[0]
    U = np.empty((B, HALF), dtype=np.float32)
    V = np.empty((B, HALF), dtype=np.float32)
    U[:, 0] = x[:, 0]
    rev = x[:, 1023:HALF:-1]
    np.add(x[:, 1:HALF], rev, out=U[:, 1:HALF])
    np.subtract(x[:, 1:HALF], rev, out=V[:, 1:HALF])
    x512 = x[:, HALF]
    a = {k: np.empty((B, QU), dtype=np.float32)
         for k in ("aue", "auo", "avo", "ave")}
    a["aue"][:, 0] = U[:, 0] + x512
    a["auo"][:, 0] = U[:, 0] - x512
    a["avo"][:, 0] = 0.0                       # dead slot (EI E-row 0 is 0)
    a["ave"][:, 0] = V[:, QU]                  # rides OI E-row-0 pattern
    urev = U[:, 511:QU:-1]
    vrev = V[:, 511:QU:-1]
    np.add(U[:, 1:QU], urev, out=a["aue"][:, 1:QU])
    np.subtract(U[:, 1:QU], urev, out=a["auo"][:, 1:QU])
    np.subtract(V[:, 1:QU], vrev, out=a["avo"][:, 1:QU])
    np.add(V[:, 1:QU], vrev, out=a["ave"][:, 1:QU])
    col0 = (U.sum(axis=1, dtype=np.float64) + x512).astype(np.float32)
    u256 = U[:, QU].copy()                     # x[256] + x[768]
    # Xr[512] = sum x[even] - sum x[odd]
    xr512 = (x[:, 0::2].sum(axis=1, dtype=np.float64)
             - x[:, 1::2].sum(axis=1, dtype=np.float64)).astype(np.float32)
    at = {k: np.ascontiguousarray(v.T, dtype=np.float16)   # [256, B] fp16
          for k, v in a.items()}
    return at, col0, u256, xr512


def _blocked(a_t, sl, b_core):
    """[256, B] fp16 column-slice -> [128(p), n_groups, 2(j), gc]."""
    gc = min(GC, b_core)
    n_groups = b_core // gc
    s = a_t[:, sl].reshape(P, 2, n_groups, gc)          # [p, j, g, b]
    return np.ascontiguousarray(s.transpose(0, 2, 1, 3))  # [p, g, j, b]


def _assemble(half_t, out, sl, b_core, is_imag, u256=None, xr512=None):
    """Device half [n_groups, 512(r=4p+slot), gc] -> out[sl, :] (1024 cols).

    slots (0,1) = (E,O) of the even-k quadrant, (2,3) = (E,O) of odd-k.
    Even-k butterfly pairs c <-> 254-c (freq 2(c+1) <-> 2(255-c); col
    255 = freq 512 handled on host); odd-k pairs c <-> 255-c (freq
    2c+1 <-> 2(255-c)+1). Imag halves flip the sign at the mirrored
    column.
    """
    gc = min(GC, b_core)
    n_groups = b_core // gc
    h = half_t.reshape(n_groups, P, 4, gc)
    b0 = sl.start
    msign = -1.0 if is_imag else 1.0
    for g in range(n_groups):
        rows = slice(b0 + g * gc, b0 + (g + 1) * gc)
        blk = out[rows]
        # even-k quadrant: freqs 2,4,..,256 then partners 510,508,..,258
        e = h[g, :, 0, :].astype(np.float32)   # [128, gc]
        o = h[g, :, 1, :].astype(np.float32)
        blk[:, 2:258:2] = (e + o).T
        blk[:, 510:256:-2] = (msign * (e - o)[:127]).T
        # odd-k quadrant: freqs 1,3,..,255 then partners 511,509,..,257
        e = h[g, :, 2, :].astype(np.float32)
        o = h[g, :, 3, :].astype(np.float32)
        blk[:, 1:257:2] = (e + o).T
        blk[:, 511:255:-2] = (msign * (e - o)).T
        if is_imag:
            blk[:, 512] = 0.0
        else:
            # rank-1 U[256] correction on even freqs 2..510: sign
            # (-1)^(c+1) at freq 2(c+1) -> -1 at freq 4k+2, +1 at 4k
            u = u256[rows]
            blk[:, 2:512:4] -= u[:, None]
            blk[:, 4:512:4] += u[:, None]
            blk[:, 512] = xr512[rows]
    blk = out[sl]
    if is_imag:
        np.negative(blk[:, 511:0:-1], out=blk[:, 513:1024])
    else:
        blk[:, 513:1024] = blk[:, 511:0:-1]


def kernel(**inputs):
    from concourse.bass_utils import run_bass_kernel_spmd

    x = np.ascontiguousarray(np.asarray(inputs["x"], dtype=np.float32))
    assert x.shape == (BATCH, N_FFT), x.shape
    coef = _constants()
    at, col0, u256, xr512 = _host_prep(x)
    nc = _get_nc()
    gc = min(GC, B_CORE)
    n_groups = B_CORE // gc
    names = ("aue", "avo", "auo", "ave")
    in_maps = []
    for c in range(N_CORES):
        sl = slice(c * B_CORE, (c + 1) * B_CORE)
        bl = {k: _blocked(v, sl, B_CORE) for k, v in at.items()}
        # pack all data in stream order: slot s = 4g + array
        dd = np.stack(
            [bl[k][:, g] for g in range(n_groups) for k in names], axis=1
        )
        m = {"dd": np.ascontiguousarray(dd), "coef": coef}
        in_maps.append(m)
    trace = bool(int(os.environ.get("FFT_KERNEL_TRACE", "0")))

    def _run():
        return run_bass_kernel_spmd(
            nc, in_maps, core_ids=list(range(N_CORES)), trace=trace
        )

    def _corrupt(r):
        # outputs are bounded well inside fp16 range, so any non-finite
        # value means a transient device/DMA corruption -> retry
        return any(
            not np.isfinite(np.asarray(r.results[c][t], dtype=np.float32)).all()
            for c in range(N_CORES)
            for t in ("o_rt", "o_it")
        )

    try:
        res = _run()
        if _corrupt(res):
            res = _run()
    except Exception:
        # transient NRT/device hiccups have been observed; retry once
        res = _run()
    if trace:
        kernel.last_results = res
    real = np.empty((BATCH, N_FFT), dtype=np.float32)
    imag = np.empty((BATCH, N_FFT), dtype=np.float32)
    for c in range(N_CORES):
        sl = slice(c * B_CORE, (c + 1) * B_CORE)
        _assemble(res.results[c]["o_rt"], real, sl, B_CORE, is_imag=False,
                  u256=u256, xr512=xr512)
        _assemble(res.results[c]["o_it"], imag, sl, B_CORE, is_imag=True)
    real[:, 0] = col0
    imag[:, 0] = 0.0
    return real, imag


# revision 22
# speedup vs baseline: 1.0309x; 1.0309x over previous
"""Trainium2 Bass kernel: 1024-point FFT of real rows -> (real, imag).

Math: out = FFT_1024(x[b, :]) per row. Two folding levels over the real
input x (U[n] = x[n]+x[1024-n], V[n] = x[n]-x[1024-n]) give four real
arrays Aue/Auo/Avo/Ave of length 256 per row whose cos/sin transforms
are the four spectrum quadrants (even/odd k x real/imag):
  Xr[2m]   = Aue @ cos(2pi n m/512)   + U[256](-1)^m
  Xi[2m]   = Avo @ (-sin(2pi n m/512))
  Xr[2m+1] = Auo @ cos(pi n(2m+1)/512)
  Xi[2m+1] = Ave @ (-sin(pi n(2m+1)/512)) - V[256](-1)^m
A third radix-2 level halves the device matmul work: each quadrant
matrix C satisfies C[n, sigma(c)] = +/-(-1)^n C[n, c] for a column
involution sigma (sigma(c) = 254-c for even-k quadrants, 255-c for
odd-k), so splitting the data by row parity gives two [128x128]
products E = A[0::2] @ C[0::2, :128], O = A[1::2] @ C[1::2, :128]
and the host reconstructs both column halves with one butterfly:
quad[c] = E+O, quad[sigma(c)] = +/-(E-O). Edge terms (the U[256]
rank-1 term, k=0, k=512) are host-side rank-1 corrections; the
V[256](-1)^m term rides row 0 of the OI even matrix, which is
naturally zero. PE work per row: 8 x 128x128 MACs — half the naive
quadrant cost — so the tensor engine never paces the kernel.

All device I/O is fp16 (the kernel is HBM-bandwidth-bound with a
shared ~420 B/ns read+write cap; fp16 of folded data + coefficients
keeps L2 relative error ~4e-4). Data ships as one packed tensor in
stream order, DMA'd as per-(group, array) 512KB chunks (row n = 2p+j
is already the parity split) so each quadrant's matmuls start the
moment its chunk lands and output drains interleave with the input
stream. The 8 quarter-size matrices ship packed [128, 8, 128]
(2KB/partition, one 256KB DMA, first in the stream). Per group the
quadrants run in input-arrival order (ER, EI, OR, OI); each
sub-product is a single K=128 matmul into its own 512-col PSUM bank
(E/O tags x 2 bufs + 1 spin bank), h0 halves convert on vector and h1
on scalar, and each quadrant's (E, O) slot pair is adjacent DRAM rows
drained on the gpsimd queue as soon as its copies land (the final
group drains per-slot, the last quadrant on the idle sync queue, to
smooth the DMA tail). ~8.7MB/core total HBM traffic.

The chip throttles HBM bandwidth and engine clocks ~50% until a
power-state promotion that follows a few us of sustained
tensor-engine activity, and demotes again shortly after the PE goes
idle — so the PE spins wide zero matmuls from the first possible
cycle until real operands arrive, in short bursts between quadrants,
and through the final copy/drain tail (sized to end before the last
drain so exec time is never extended).

The host performs the pure data-expansion assembly: butterflies,
parity interleave, conjugate mirror, k=0/512 columns, rank-1 edge
corrections, final transpose, fp16->fp32 upcast. Pure data-parallel
across 8 cores, no collectives.
"""

import os
import numpy as np

N_FFT = 1024
BATCH = 16384
N_CORES = 8
B_CORE = BATCH // N_CORES  # 2048
P = 128
HALF = 512
QU = 256
GC = 1024                  # batch rows per group

_BUILD_CACHE = {}


def _constants():
    """8 packed [K=128, M=128] fp16 matrices: (E, O) per quadrant.

    packed[p, 2q+part, c] = Cq[2p+part, c] for c in [0, 128); row 0 of
    the OI even matrix (naturally zero) carries the -V[256](-1)^m term
    as the pattern (-1)^(c+1).
    """
    n = np.arange(QU, dtype=np.float64)[:, None]
    c = np.arange(QU, dtype=np.float64)[None, :]
    cer = np.cos(2 * np.pi * n * (c + 1) / 512)
    cei = -np.sin(2 * np.pi * n * (c + 1) / 512)
    cor = np.cos(np.pi * n * (2 * c + 1) / 512)
    coi = -np.sin(np.pi * n * (2 * c + 1) / 512)
    coi_e = coi[0::2, :P].copy()
    coi_e[0, :] = (-1.0) ** (np.arange(P) + 1)   # V[256] edge pattern
    mats = [
        cer[0::2, :P], cer[1::2, :P],   # ER: aue
        cei[0::2, :P], cei[1::2, :P],   # EI: avo (row 0 of E is zero)
        cor[0::2, :P], cor[1::2, :P],   # OR: auo
        coi_e, coi[1::2, :P],           # OI: ave
    ]
    packed = np.stack(mats, axis=1).astype(np.float16)  # [128, 8, 128]
    return np.ascontiguousarray(packed)


def build_nc(b_core=B_CORE):
    """Build + compile the per-core Bass program (same NEFF on all cores)."""
    import concourse.mybir as mybir
    import concourse.tile as tile
    from concourse import bacc

    f16 = mybir.dt.float16
    f32 = mybir.dt.float32

    gc = min(GC, b_core)
    n_groups = b_core // gc
    n_h = gc // HALF           # 512-col PSUM halves per group (2)

    nc = bacc.Bacc(
        "TRN2", target_bir_lowering=False, debug=False, num_devices=N_CORES
    )

    # all folded data packed in stream order: slot s = 4g + array,
    # arrays ordered (aue, avo, auo, ave) = quadrant processing order
    data_in = nc.dram_tensor(
        "dd", [P, 4 * n_groups, 2, gc], f16, kind="ExternalInput"
    )
    coef_in = nc.dram_tensor("coef", [P, 8, P], f16, kind="ExternalInput")
    # transposed halves, group-blocked: row r = 4p + slot;
    # rt slots: [ER_E, ER_O, OR_E, OR_O]; it slots: [EI_E, EI_O, OI_E, OI_O]
    o_rt = nc.dram_tensor("o_rt", [n_groups, 2 * QU, gc], f16, kind="ExternalOutput")
    o_it = nc.dram_tensor("o_it", [n_groups, 2 * QU, gc], f16, kind="ExternalOutput")

    ort_r = o_rt.ap().rearrange("g (p t) b -> g p t b", t=4)
    oit_r = o_it.ap().rearrange("g (p t) b -> g p t b", t=4)

    with tile.TileContext(nc) as tc:
        with (
            tc.tile_pool(name="const", bufs=1) as cpool,
            tc.tile_pool(name="work", bufs=1) as wpool,
            tc.tile_pool(name="outp", bufs=2) as opool,
            tc.tile_pool(name="psm", bufs=1, space="PSUM") as psm,
        ):
            coef_sb = cpool.tile([P, 8, P], f16)
            nc.sync.dma_start(out=coef_sb[:], in_=coef_in.ap())
            # per-(group, array) 512KB chunks in stream order: each
            # quadrant's matmuls start the moment its chunk lands, so
            # output drains interleave with the input stream
            dat = wpool.tile([P, 4 * n_groups, 2, gc], f16, name="dat")
            for s in range(4 * n_groups):
                nc.sync.dma_start(out=dat[:, s], in_=data_in.ap()[:, s])

            # PE-activity warmup: HBM bandwidth and engine clocks are capped
            # ~50% until a power-state promotion that follows a few us of
            # sustained tensor-engine activity (and decays again shortly
            # after it stops), so spin wide matmuls from the first possible
            # cycle until the first data chunk lands, and keep short spin
            # bursts between real matmuls later (dedicated PSUM bank, zero
            # data deps) to hold the promoted state through the write tail.
            wu_in = cpool.tile([P, 4 * P], f16)
            nc.gpsimd.memset(wu_in[:], 0.0)
            wu = psm.tile([P, HALF], f32, tag="SPIN", bufs=1)
            n_wu = 11
            for w in range(n_wu):
                nc.tensor.matmul(
                    wu[:], lhsT=wu_in[:, 0:P], rhs=wu_in[:, 0:HALF],
                    start=(w == 0), stop=(w == n_wu - 1),
                )

            def spin(n):
                for _ in range(n):
                    nc.tensor.matmul(
                        wu[:], lhsT=wu_in[:, 0:P], rhs=wu_in[:, 0:HALF],
                        start=True, stop=True,
                    )

            for g in range(n_groups):
                ortg = opool.tile([P, 4, gc], f16, tag="ortg")
                oitg = opool.tile([P, 4, gc], f16, tag="oitg")

                # quadrant-major order matches input arrival
                # (aue -> avo -> auo -> ave); the (E, O) slot pair is
                # adjacent DRAM rows drained together.
                last = g == n_groups - 1
                for qi in range(4):
                    stage, st_r = (
                        (ortg, ort_r) if qi % 2 == 0 else (oitg, oit_r)
                    )
                    base = 0 if qi < 2 else 2
                    # per-512-col-half PSUM tiles (1 bank each): the half's
                    # copy starts while the next half's matmul runs, and
                    # E/O × 2 bufs leave a free bank for the spin tile.
                    # h0 halves convert on vector, h1 on scalar.
                    for part in range(2):
                        tagp = "E" if part == 0 else "O"
                        for h in range(n_h):
                            bsl = slice(h * HALF, (h + 1) * HALF)
                            ph = psm.tile([P, HALF], f32, tag=tagp, bufs=2)
                            nc.tensor.matmul(
                                ph[:], lhsT=coef_sb[:, 2 * qi + part],
                                rhs=dat[:, 4 * g + qi, part, bsl],
                                start=True, stop=True,
                            )
                            if h == 0:
                                nc.vector.tensor_copy(
                                    out=stage[:, base + part, bsl], in_=ph[:]
                                )
                            else:
                                nc.scalar.copy(
                                    out=stage[:, base + part, bsl], in_=ph[:]
                                )
                    spin(2)
                    if last and qi >= 2:
                        # final group's last quadrants: drain each slot as
                        # soon as its copies land; the OI pair rides the
                        # (now idle) lower-latency sync queue
                        eng = nc.gpsimd if qi == 2 else nc.sync
                        eng.dma_start(
                            out=st_r[g][:, base : base + 1],
                            in_=stage[:, base : base + 1],
                        )
                        eng.dma_start(
                            out=st_r[g][:, base + 1 : base + 2],
                            in_=stage[:, base + 1 : base + 2],
                        )
                    else:
                        nc.gpsimd.dma_start(
                            out=st_r[g][:, base : base + 2],
                            in_=stage[:, base : base + 2],
                        )

            # tail spins: keep the PE (and the power state) hot while the
            # final copies convert and drain; sized to finish just before
            # the last drain lands so exec time is not extended
            spin(16)

    nc.compile()
    return nc


def _get_nc(b_core=B_CORE):
    if b_core not in _BUILD_CACHE:
        _BUILD_CACHE[b_core] = build_nc(b_core)
    return _BUILD_CACHE[b_core]


def _host_prep(x):
    """Two-level real-FFT folds (transposed) + host-side edge columns."""
    B = x.shape[0]
    U = np.empty((B, HALF), dtype=np.float32)
    V = np.empty((B, HALF), dtype=np.float32)
    U[:, 0] = x[:, 0]
    rev = x[:, 1023:HALF:-1]
    np.add(x[:, 1:HALF], rev, out=U[:, 1:HALF])
    np.subtract(x[:, 1:HALF], rev, out=V[:, 1:HALF])
    x512 = x[:, HALF]
    a = {k: np.empty((B, QU), dtype=np.float32)
         for k in ("aue", "auo", "avo", "ave")}
    a["aue"][:, 0] = U[:, 0] + x512
    a["auo"][:, 0] = U[:, 0] - x512
    a["avo"][:, 0] = 0.0                       # dead slot (EI E-row 0 is 0)
    a["ave"][:, 0] = V[:, QU]                  # rides OI E-row-0 pattern
    urev = U[:, 511:QU:-1]
    vrev = V[:, 511:QU:-1]
    np.add(U[:, 1:QU], urev, out=a["aue"][:, 1:QU])
    np.subtract(U[:, 1:QU], urev, out=a["auo"][:, 1:QU])
    np.subtract(V[:, 1:QU], vrev, out=a["avo"][:, 1:QU])
    np.add(V[:, 1:QU], vrev, out=a["ave"][:, 1:QU])
    col0 = (U.sum(axis=1, dtype=np.float64) + x512).astype(np.float32)
    u256 = U[:, QU].copy()                     # x[256] + x[768]
    # Xr[512] = sum x[even] - sum x[odd]
    xr512 = (x[:, 0::2].sum(axis=1, dtype=np.float64)
             - x[:, 1::2].sum(axis=1, dtype=np.float64)).astype(np.float32)
    at = {k: np.ascontiguousarray(v.T, dtype=np.float16)   # [256, B] fp16
          for k, v in a.items()}
    return at, col0, u256, xr512


def _blocked(a_t, sl, b_core):
    """[256, B] fp16 column-slice -> [128(p), n_groups, 2(j), gc]."""
    gc = min(GC, b_core)
    n_groups = b_core // gc
    s = a_t[:, sl].reshape(P, 2, n_groups, gc)          # [p, j, g, b]
    return np.ascontiguousarray(s.transpose(0, 2, 1, 3))  # [p, g, j, b]


def _assemble(half_t, out, sl, b_core, is_imag, u256=None, xr512=None):
    """Device half [n_groups, 512(r=4p+slot), gc] -> out[sl, :] (1024 cols).

    slots (0,1) = (E,O) of the even-k quadrant, (2,3) = (E,O) of odd-k.
    Even-k butterfly pairs c <-> 254-c (freq 2(c+1) <-> 2(255-c); col
    255 = freq 512 handled on host); odd-k pairs c <-> 255-c (freq
    2c+1 <-> 2(255-c)+1). Imag halves flip the sign at the mirrored
    column.
    """
    gc = min(GC, b_core)
    n_groups = b_core // gc
    h = half_t.reshape(n_groups, P, 4, gc)
    b0 = sl.start
    msign = -1.0 if is_imag else 1.0
    for g in range(n_groups):
        rows = slice(b0 + g * gc, b0 + (g + 1) * gc)
        blk = out[rows]
        # even-k quadrant: freqs 2,4,..,256 then partners 510,508,..,258
        e = h[g, :, 0, :].astype(np.float32)   # [128, gc]
        o = h[g, :, 1, :].astype(np.float32)
        blk[:, 2:258:2] = (e + o).T
        blk[:, 510:256:-2] = (msign * (e - o)[:127]).T
        # odd-k quadrant: freqs 1,3,..,255 then partners 511,509,..,257
        e = h[g, :, 2, :].astype(np.float32)
        o = h[g, :, 3, :].astype(np.float32)
        blk[:, 1:257:2] = (e + o).T
        blk[:, 511:255:-2] = (msign * (e - o)).T
        if is_imag:
            blk[:, 512] = 0.0
        else:
            # rank-1 U[256] correction on even freqs 2..510: sign
            # (-1)^(c+1) at freq 2(c+1) -> -1 at freq 4k+2, +1 at 4k
            u = u256[rows]
            blk[:, 2:512:4] -= u[:, None]
            blk[:, 4:512:4] += u[:, None]
            blk[:, 512] = xr512[rows]
    blk = out[sl]
    if is_imag:
        np.negative(blk[:, 511:0:-1], out=blk[:, 513:1024])
    else:
        blk[:, 513:1024] = blk[:, 511:0:-1]


def kernel(**inputs):
    from concourse.bass_utils import run_bass_kernel_spmd

    x = np.ascontiguousarray(np.asarray(inputs["x"], dtype=np.float32))
    assert x.shape == (BATCH, N_FFT), x.shape
    coef = _constants()
    at, col0, u256, xr512 = _host_prep(x)
    nc = _get_nc()
    gc = min(GC, B_CORE)
    n_groups = B_CORE // gc
    names = ("aue", "avo", "auo", "ave")
    in_maps = []
    for c in range(N_CORES):
        sl = slice(c * B_CORE, (c + 1) * B_CORE)
        bl = {k: _blocked(v, sl, B_CORE) for k, v in at.items()}
        # pack all data in stream order: slot s = 4g + array
        dd = np.stack(
            [bl[k][:, g] for g in range(n_groups) for k in names], axis=1
        )
        m = {"dd": np.ascontiguousarray(dd), "coef": coef}
        in_maps.append(m)
    trace = bool(int(os.environ.get("FFT_KERNEL_TRACE", "0")))

    def _run():
        return run_bass_kernel_spmd(
            nc, in_maps, core_ids=list(range(N_CORES)), trace=trace
        )

    def _corrupt(r):
        # outputs are bounded well inside fp16 range, so any non-finite
        # value means a transient device/DMA corruption -> retry
        return any(
            not np.isfinite(np.asarray(r.results[c][t], dtype=np.float32)).all()
            for c in range(N_CORES)
            for t in ("o_rt", "o_it")
        )

    try:
        res = _run()
        if _corrupt(res):
            res = _run()
    except Exception:
        # transient NRT/device hiccups have been observed; retry once
        res = _run()
    if trace:
        kernel.last_results = res
    real = np.empty((BATCH, N_FFT), dtype=np.float32)
    imag = np.empty((BATCH, N_FFT), dtype=np.float32)
    for c in range(N_CORES):
        sl = slice(c * B_CORE, (c + 1) * B_CORE)
        _assemble(res.results[c]["o_rt"], real, sl, B_CORE, is_imag=False,
                  u256=u256, xr512=xr512)
        _assemble(res.results[c]["o_it"], imag, sl, B_CORE, is_imag=True)
    real[:, 0] = col0
    imag[:, 0] = 0.0
    return real, imag


# revision 26
# speedup vs baseline: 1.0745x; 1.0424x over previous
"""Trainium2 Bass kernel: 1024-point FFT of real rows -> (real, imag).

Math: out = FFT_1024(x[b, :]) per row. Two folding levels over the real
input x (U[n] = x[n]+x[1024-n], V[n] = x[n]-x[1024-n]) give four real
arrays Aue/Auo/Avo/Ave of length 256 per row whose cos/sin transforms
are the four spectrum quadrants (even/odd k x real/imag):
  Xr[2m]   = Aue @ cos(2pi n m/512)   + U[256](-1)^m
  Xi[2m]   = Avo @ (-sin(2pi n m/512))
  Xr[2m+1] = Auo @ cos(pi n(2m+1)/512)
  Xi[2m+1] = Ave @ (-sin(pi n(2m+1)/512)) - V[256](-1)^m
A third radix-2 level halves the device matmul work: each quadrant
matrix C satisfies C[n, sigma(c)] = +/-(-1)^n C[n, c] for a column
involution sigma (sigma(c) = 254-c for even-k quadrants, 255-c for
odd-k), so splitting the data by row parity gives two [128x128]
products E = A[0::2] @ C[0::2, :128], O = A[1::2] @ C[1::2, :128]
and the host reconstructs both column halves with one butterfly:
quad[c] = E+O, quad[sigma(c)] = +/-(E-O). Edge terms (the U[256]
rank-1 term, k=0, k=512) are host-side rank-1 corrections; the
V[256](-1)^m term rides row 0 of the OI even matrix, which is
naturally zero. PE work per row: 8 x 128x128 MACs — half the naive
quadrant cost — so the tensor engine never paces the kernel.

All device I/O is fp16 (the kernel is HBM-bandwidth-bound with a
shared ~420 B/ns read+write cap; fp16 of folded data + coefficients
keeps L2 relative error ~4e-4). Data ships as one packed tensor in
stream order, DMA'd as per-(group, array) 512KB chunks (row n = 2p+j
is already the parity split) so each quadrant's matmuls start the
moment its chunk lands and output drains interleave with the input
stream. The 8 quarter-size matrices ship packed [128, 8, 128]
(2KB/partition, one 256KB DMA, first in the stream). Per group the
quadrants run in input-arrival order (ER, EI, OR, OI); each
sub-product is a single K=128 matmul into its own 512-col PSUM bank
(E/O tags x 2 bufs + 1 spin bank), h0 halves convert on vector and h1
on scalar, and each quadrant's (E, O) slot pair is adjacent DRAM rows
drained on the gpsimd queue as soon as its copies land (the final
group drains per-slot, the last quadrant on the idle sync queue, to
smooth the DMA tail). ~8.7MB/core total HBM traffic.

The chip throttles HBM bandwidth and engine clocks ~50% until a
power-state promotion that follows a few us of sustained
tensor-engine activity, and demotes again shortly after the PE goes
idle — so the PE spins wide zero matmuls from the first possible
cycle until real operands arrive, in short bursts between quadrants,
and through the final copy/drain tail (sized to end before the last
drain so exec time is never extended).

The host performs the pure data-expansion assembly: butterflies,
parity interleave, conjugate mirror, k=0/512 columns, rank-1 edge
corrections, final transpose, fp16->fp32 upcast. Pure data-parallel
across 8 cores, no collectives.
"""

import os
import numpy as np

N_FFT = 1024
BATCH = 16384
N_CORES = 8
B_CORE = BATCH // N_CORES  # 2048
P = 128
HALF = 512
QU = 256
GC = 1024                  # batch rows per group

_BUILD_CACHE = {}


def _constants():
    """8 packed [K=128, M=128] fp16 matrices: (E, O) per quadrant.

    packed[p, 2q+part, c] = Cq[2p+part, c] for c in [0, 128); row 0 of
    the OI even matrix (naturally zero) carries the -V[256](-1)^m term
    as the pattern (-1)^(c+1).
    """
    n = np.arange(QU, dtype=np.float64)[:, None]
    c = np.arange(QU, dtype=np.float64)[None, :]
    cer = np.cos(2 * np.pi * n * (c + 1) / 512)
    cei = -np.sin(2 * np.pi * n * (c + 1) / 512)
    cor = np.cos(np.pi * n * (2 * c + 1) / 512)
    coi = -np.sin(np.pi * n * (2 * c + 1) / 512)
    coi_e = coi[0::2, :P].copy()
    coi_e[0, :] = (-1.0) ** (np.arange(P) + 1)   # V[256] edge pattern
    mats = [
        cer[0::2, :P], cer[1::2, :P],   # ER: aue
        cei[0::2, :P], cei[1::2, :P],   # EI: avo (row 0 of E is zero)
        cor[0::2, :P], cor[1::2, :P],   # OR: auo
        coi_e, coi[1::2, :P],           # OI: ave
    ]
    packed = np.stack(mats, axis=1).astype(np.float16)  # [128, 8, 128]
    return np.ascontiguousarray(packed)


def build_nc(b_core=B_CORE):
    """Build + compile the per-core Bass program (same NEFF on all cores)."""
    import concourse.mybir as mybir
    import concourse.tile as tile
    from concourse import bacc

    f16 = mybir.dt.float16
    f32 = mybir.dt.float32

    gc = min(GC, b_core)
    n_groups = b_core // gc
    n_h = gc // HALF           # 512-col PSUM halves per group (2)

    nc = bacc.Bacc(
        "TRN2", target_bir_lowering=False, debug=False, num_devices=N_CORES
    )

    # all folded data packed in stream order: slot s = 4g + array,
    # arrays ordered (aue, avo, auo, ave) = quadrant processing order
    data_in = nc.dram_tensor(
        "dd", [P, 4 * n_groups, 2, gc], f16, kind="ExternalInput"
    )
    coef_in = nc.dram_tensor("coef", [P, 8, P], f16, kind="ExternalInput")
    # transposed halves, group-blocked: row r = 4p + slot;
    # rt slots: [ER_E, ER_O, OR_E, OR_O]; it slots: [EI_E, EI_O, OI_E, OI_O]
    o_rt = nc.dram_tensor("o_rt", [n_groups, 2 * QU, gc], f16, kind="ExternalOutput")
    o_it = nc.dram_tensor("o_it", [n_groups, 2 * QU, gc], f16, kind="ExternalOutput")

    ort_r = o_rt.ap().rearrange("g (p t) b -> g p t b", t=4)
    oit_r = o_it.ap().rearrange("g (p t) b -> g p t b", t=4)

    with tile.TileContext(nc) as tc:
        with (
            tc.tile_pool(name="const", bufs=1) as cpool,
            tc.tile_pool(name="work", bufs=1) as wpool,
            tc.tile_pool(name="outp", bufs=2) as opool,
            tc.tile_pool(name="psm", bufs=1, space="PSUM") as psm,
        ):
            coef_sb = cpool.tile([P, 8, P], f16)
            nc.sync.dma_start(out=coef_sb[:], in_=coef_in.ap())
            # per-(group, array) 512KB chunks in stream order: each
            # quadrant's matmuls start the moment its chunk lands, so
            # output drains interleave with the input stream
            dat = wpool.tile([P, 4 * n_groups, 2, gc], f16, name="dat")
            for s in range(4 * n_groups):
                nc.sync.dma_start(out=dat[:, s], in_=data_in.ap()[:, s])

            # PE-activity warmup: HBM bandwidth and engine clocks are capped
            # ~50% until a power-state promotion that follows a few us of
            # sustained tensor-engine activity (and decays again shortly
            # after it stops), so spin wide matmuls from the first possible
            # cycle until the first data chunk lands, and keep short spin
            # bursts between real matmuls later (dedicated PSUM bank, zero
            # data deps) to hold the promoted state through the write tail.
            wu_in = cpool.tile([P, 4 * P], f16)
            nc.gpsimd.memset(wu_in[:], 0.0)
            wu = psm.tile([P, HALF], f32, tag="SPIN", bufs=1)
            n_wu = 11
            for w in range(n_wu):
                nc.tensor.matmul(
                    wu[:], lhsT=wu_in[:, 0:P], rhs=wu_in[:, 0:HALF],
                    start=(w == 0), stop=(w == n_wu - 1),
                )

            def spin(n):
                for _ in range(n):
                    nc.tensor.matmul(
                        wu[:], lhsT=wu_in[:, 0:P], rhs=wu_in[:, 0:HALF],
                        start=True, stop=True,
                    )

            for g in range(n_groups):
                ortg = opool.tile([P, 4, gc], f16, tag="ortg")
                oitg = opool.tile([P, 4, gc], f16, tag="oitg")

                # quadrant-major order matches input arrival
                # (aue -> avo -> auo -> ave); the (E, O) slot pair is
                # adjacent DRAM rows drained together.
                last = g == n_groups - 1
                for qi in range(4):
                    stage, st_r = (
                        (ortg, ort_r) if qi % 2 == 0 else (oitg, oit_r)
                    )
                    base = 0 if qi < 2 else 2
                    # per-512-col-half PSUM tiles (1 bank each): the half's
                    # copy starts while the next half's matmul runs, and
                    # E/O × 2 bufs leave a free bank for the spin tile.
                    # h0 halves convert on vector, h1 on scalar.
                    for part in range(2):
                        tagp = "E" if part == 0 else "O"
                        for h in range(n_h):
                            bsl = slice(h * HALF, (h + 1) * HALF)
                            ph = psm.tile([P, HALF], f32, tag=tagp, bufs=2)
                            nc.tensor.matmul(
                                ph[:], lhsT=coef_sb[:, 2 * qi + part],
                                rhs=dat[:, 4 * g + qi, part, bsl],
                                start=True, stop=True,
                            )
                            if h == 0:
                                nc.vector.tensor_copy(
                                    out=stage[:, base + part, bsl], in_=ph[:]
                                )
                            else:
                                nc.scalar.copy(
                                    out=stage[:, base + part, bsl], in_=ph[:]
                                )
                    spin(2)
                    if last and qi >= 2:
                        # final group's last quadrants: drain each slot as
                        # soon as its copies land; the OI pair rides the
                        # (now idle) lower-latency sync queue
                        eng = nc.gpsimd if qi == 2 else nc.sync
                        eng.dma_start(
                            out=st_r[g][:, base : base + 1],
                            in_=stage[:, base : base + 1],
                        )
                        eng.dma_start(
                            out=st_r[g][:, base + 1 : base + 2],
                            in_=stage[:, base + 1 : base + 2],
                        )
                    else:
                        nc.gpsimd.dma_start(
                            out=st_r[g][:, base : base + 2],
                            in_=stage[:, base : base + 2],
                        )

            # tail spins: keep the PE (and the power state) hot while the
            # final copies convert and drain; sized to finish just before
            # the last drain lands so exec time is not extended
            spin(16)

    nc.compile()
    return nc


def _get_nc(b_core=B_CORE):
    if b_core not in _BUILD_CACHE:
        _BUILD_CACHE[b_core] = build_nc(b_core)
    return _BUILD_CACHE[b_core]


def _host_prep(x):
    """Two-level real-FFT folds (transposed) + host-side edge columns."""
    B = x.shape[0]
    U = np.empty((B, HALF), dtype=np.float32)
    V = np.empty((B, HALF), dtype=np.float32)
    U[:, 0] = x[:, 0]
    rev = x[:, 1023:HALF:-1]
    np.add(x[:, 1:HALF], rev, out=U[:, 1:HALF])
    np.subtract(x[:, 1:HALF], rev, out=V[:, 1:HALF])
    x512 = x[:, HALF]
    a = {k: np.empty((B, QU), dtype=np.float32)
         for k in ("aue", "auo", "avo", "ave")}
    a["aue"][:, 0] = U[:, 0] + x512
    a["auo"][:, 0] = U[:, 0] - x512
    a["avo"][:, 0] = 0.0                       # dead slot (EI E-row 0 is 0)
    a["ave"][:, 0] = V[:, QU]                  # rides OI E-row-0 pattern
    urev = U[:, 511:QU:-1]
    vrev = V[:, 511:QU:-1]
    np.add(U[:, 1:QU], urev, out=a["aue"][:, 1:QU])
    np.subtract(U[:, 1:QU], urev, out=a["auo"][:, 1:QU])
    np.subtract(V[:, 1:QU], vrev, out=a["avo"][:, 1:QU])
    np.add(V[:, 1:QU], vrev, out=a["ave"][:, 1:QU])
    col0 = (U.sum(axis=1, dtype=np.float64) + x512).astype(np.float32)
    u256 = U[:, QU].copy()                     # x[256] + x[768]
    # Xr[512] = sum x[even] - sum x[odd]
    xr512 = (x[:, 0::2].sum(axis=1, dtype=np.float64)
             - x[:, 1::2].sum(axis=1, dtype=np.float64)).astype(np.float32)
    at = {k: np.ascontiguousarray(v.T, dtype=np.float16)   # [256, B] fp16
          for k, v in a.items()}
    return at, col0, u256, xr512


def _blocked(a_t, sl, b_core):
    """[256, B] fp16 column-slice -> [128(p), n_groups, 2(j), gc]."""
    gc = min(GC, b_core)
    n_groups = b_core // gc
    s = a_t[:, sl].reshape(P, 2, n_groups, gc)          # [p, j, g, b]
    return np.ascontiguousarray(s.transpose(0, 2, 1, 3))  # [p, g, j, b]


def _assemble(half_t, out, sl, b_core, is_imag, u256=None, xr512=None):
    """Device half [n_groups, 512(r=4p+slot), gc] -> out[sl, :] (1024 cols).

    slots (0,1) = (E,O) of the even-k quadrant, (2,3) = (E,O) of odd-k.
    Even-k butterfly pairs c <-> 254-c (freq 2(c+1) <-> 2(255-c); col
    255 = freq 512 handled on host); odd-k pairs c <-> 255-c (freq
    2c+1 <-> 2(255-c)+1). Imag halves flip the sign at the mirrored
    column.
    """
    gc = min(GC, b_core)
    n_groups = b_core // gc
    h = half_t.reshape(n_groups, P, 4, gc)
    b0 = sl.start
    msign = -1.0 if is_imag else 1.0
    for g in range(n_groups):
        rows = slice(b0 + g * gc, b0 + (g + 1) * gc)
        blk = out[rows]
        # even-k quadrant: freqs 2,4,..,256 then partners 510,508,..,258
        e = h[g, :, 0, :].astype(np.float32)   # [128, gc]
        o = h[g, :, 1, :].astype(np.float32)
        blk[:, 2:258:2] = (e + o).T
        blk[:, 510:256:-2] = (msign * (e - o)[:127]).T
        # odd-k quadrant: freqs 1,3,..,255 then partners 511,509,..,257
        e = h[g, :, 2, :].astype(np.float32)
        o = h[g, :, 3, :].astype(np.float32)
        blk[:, 1:257:2] = (e + o).T
        blk[:, 511:255:-2] = (msign * (e - o)).T
        if is_imag:
            blk[:, 512] = 0.0
        else:
            # rank-1 U[256] correction on even freqs 2..510: sign
            # (-1)^(c+1) at freq 2(c+1) -> -1 at freq 4k+2, +1 at 4k
            u = u256[rows]
            blk[:, 2:512:4] -= u[:, None]
            blk[:, 4:512:4] += u[:, None]
            blk[:, 512] = xr512[rows]
    blk = out[sl]
    if is_imag:
        np.negative(blk[:, 511:0:-1], out=blk[:, 513:1024])
    else:
        blk[:, 513:1024] = blk[:, 511:0:-1]


def kernel(**inputs):
    from concourse.bass_utils import run_bass_kernel_spmd

    x = np.ascontiguousarray(np.asarray(inputs["x"], dtype=np.float32))
    assert x.shape == (BATCH, N_FFT), x.shape
    coef = _constants()
    at, col0, u256, xr512 = _host_prep(x)
    nc = _get_nc()
    gc = min(GC, B_CORE)
    n_groups = B_CORE // gc
    names = ("aue", "avo", "auo", "ave")
    in_maps = []
    for c in range(N_CORES):
        sl = slice(c * B_CORE, (c + 1) * B_CORE)
        bl = {k: _blocked(v, sl, B_CORE) for k, v in at.items()}
        # pack all data in stream order: slot s = 4g + array
        dd = np.stack(
            [bl[k][:, g] for g in range(n_groups) for k in names], axis=1
        )
        m = {"dd": np.ascontiguousarray(dd), "coef": coef}
        in_maps.append(m)
    trace = bool(int(os.environ.get("FFT_KERNEL_TRACE", "0")))

    def _run():
        return run_bass_kernel_spmd(
            nc, in_maps, core_ids=list(range(N_CORES)), trace=trace
        )

    def _corrupt(r):
        # outputs are bounded well inside fp16 range, so any non-finite
        # value means a transient device/DMA corruption -> retry
        return any(
            not np.isfinite(np.asarray(r.results[c][t], dtype=np.float32)).all()
            for c in range(N_CORES)
            for t in ("o_rt", "o_it")
        )

    try:
        res = _run()
        if _corrupt(res):
            res = _run()
    except Exception:
        # transient NRT/device hiccups have been observed; retry once
        res = _run()
    if trace:
        kernel.last_results = res
    real = np.empty((BATCH, N_FFT), dtype=np.float32)
    imag = np.empty((BATCH, N_FFT), dtype=np.float32)
    for c in range(N_CORES):
        sl = slice(c * B_CORE, (c + 1) * B_CORE)
        _assemble(res.results[c]["o_rt"], real, sl, B_CORE, is_imag=False,
                  u256=u256, xr512=xr512)
        _assemble(res.results[c]["o_it"], imag, sl, B_CORE, is_imag=True)
    real[:, 0] = col0
    imag[:, 0] = 0.0
    return real, imag


# revision 27
# speedup vs baseline: 1.1373x; 1.0584x over previous
"""Trainium2 Bass kernel: 1024-point FFT of real rows -> (real, imag).

Math: out = FFT_1024(x[b, :]) per row. Two folding levels over the real
input x (U[n] = x[n]+x[1024-n], V[n] = x[n]-x[1024-n]) give four real
arrays Aue/Auo/Avo/Ave of length 256 per row whose cos/sin transforms
are the four spectrum quadrants (even/odd k x real/imag):
  Xr[2m]   = Aue @ cos(2pi n m/512)   + U[256](-1)^m
  Xi[2m]   = Avo @ (-sin(2pi n m/512))
  Xr[2m+1] = Auo @ cos(pi n(2m+1)/512)
  Xi[2m+1] = Ave @ (-sin(pi n(2m+1)/512)) - V[256](-1)^m
A third radix-2 level halves the device matmul work: each quadrant
matrix C satisfies C[n, sigma(c)] = +/-(-1)^n C[n, c] for a column
involution sigma (sigma(c) = 254-c for even-k quadrants, 255-c for
odd-k), so splitting the data by row parity gives two [128x128]
products E = A[0::2] @ C[0::2, :128], O = A[1::2] @ C[1::2, :128]
and the host reconstructs both column halves with one butterfly:
quad[c] = E+O, quad[sigma(c)] = +/-(E-O). Edge terms (the U[256]
rank-1 term, k=0, k=512) are host-side rank-1 corrections; the
V[256](-1)^m term rides row 0 of the OI even matrix, which is
naturally zero. PE work per row: 8 x 128x128 MACs — half the naive
quadrant cost — so the tensor engine never paces the kernel.

All device I/O is fp16 (the kernel is HBM-bandwidth-bound with a
shared ~420 B/ns read+write cap; fp16 of folded data + coefficients
keeps L2 relative error ~4e-4). Data ships as one packed tensor in
stream order, DMA'd as per-(group, array) 512KB chunks (row n = 2p+j
is already the parity split) so each quadrant's matmuls start the
moment its chunk lands and output drains interleave with the input
stream. The 8 quarter-size matrices ship packed [128, 8, 128]
(2KB/partition, one 256KB DMA, first in the stream). Per group the
quadrants run in input-arrival order (ER, EI, OR, OI); each
sub-product is a single K=128 matmul into its own 512-col PSUM bank
(E/O tags x 2 bufs + 1 spin bank), h0 halves convert on vector and h1
on scalar, and each quadrant's (E, O) slot pair is adjacent DRAM rows
drained on the gpsimd queue as soon as its copies land (the final
group drains per-slot, the last quadrant on the idle sync queue, to
smooth the DMA tail). ~8.7MB/core total HBM traffic.

The chip throttles HBM bandwidth and engine clocks ~50% until a
power-state promotion that follows a few us of sustained
tensor-engine activity, and demotes again shortly after the PE goes
idle — so the PE spins wide zero matmuls from the first possible
cycle until real operands arrive, in short bursts between quadrants,
and through the final copy/drain tail (sized to end before the last
drain so exec time is never extended).

The host performs the pure data-expansion assembly: butterflies,
parity interleave, conjugate mirror, k=0/512 columns, rank-1 edge
corrections, final transpose, fp16->fp32 upcast. Pure data-parallel
across 8 cores, no collectives.
"""

import os
import numpy as np

N_FFT = 1024
BATCH = 16384
N_CORES = 8
B_CORE = BATCH // N_CORES  # 2048
P = 128
HALF = 512
QU = 256
GC = 1024                  # batch rows per group

_BUILD_CACHE = {}


def _constants():
    """8 packed [K=128, M=128] fp16 matrices: (E, O) per quadrant.

    packed[p, 2q+part, c] = Cq[2p+part, c] for c in [0, 128); row 0 of
    the OI even matrix (naturally zero) carries the -V[256](-1)^m term
    as the pattern (-1)^(c+1).
    """
    n = np.arange(QU, dtype=np.float64)[:, None]
    c = np.arange(QU, dtype=np.float64)[None, :]
    cer = np.cos(2 * np.pi * n * (c + 1) / 512)
    cei = -np.sin(2 * np.pi * n * (c + 1) / 512)
    cor = np.cos(np.pi * n * (2 * c + 1) / 512)
    coi = -np.sin(np.pi * n * (2 * c + 1) / 512)
    coi_e = coi[0::2, :P].copy()
    coi_e[0, :] = (-1.0) ** (np.arange(P) + 1)   # V[256] edge pattern
    mats = [
        cer[0::2, :P], cer[1::2, :P],   # ER: aue
        cei[0::2, :P], cei[1::2, :P],   # EI: avo (row 0 of E is zero)
        cor[0::2, :P], cor[1::2, :P],   # OR: auo
        coi_e, coi[1::2, :P],           # OI: ave
    ]
    packed = np.stack(mats, axis=1).astype(np.float16)  # [128, 8, 128]
    return np.ascontiguousarray(packed)


def build_nc(b_core=B_CORE):
    """Build + compile the per-core Bass program (same NEFF on all cores)."""
    import concourse.mybir as mybir
    import concourse.tile as tile
    from concourse import bacc

    f16 = mybir.dt.float16
    f32 = mybir.dt.float32

    gc = min(GC, b_core)
    n_groups = b_core // gc
    n_h = gc // HALF           # 512-col PSUM halves per group (2)

    nc = bacc.Bacc(
        "TRN2", target_bir_lowering=False, debug=False, num_devices=N_CORES
    )

    # all folded data packed in stream order: slot s = 4g + array,
    # arrays ordered (aue, avo, auo, ave) = quadrant processing order
    data_in = nc.dram_tensor(
        "dd", [P, 4 * n_groups, 2, gc], f16, kind="ExternalInput"
    )
    coef_in = nc.dram_tensor("coef", [P, 8, P], f16, kind="ExternalInput")
    # transposed halves, group-blocked: row r = 4p + slot;
    # rt slots: [ER_E, ER_O, OR_E, OR_O]; it slots: [EI_E, EI_O, OI_E, OI_O]
    o_rt = nc.dram_tensor("o_rt", [n_groups, 2 * QU, gc], f16, kind="ExternalOutput")
    o_it = nc.dram_tensor("o_it", [n_groups, 2 * QU, gc], f16, kind="ExternalOutput")

    ort_r = o_rt.ap().rearrange("g (p t) b -> g p t b", t=4)
    oit_r = o_it.ap().rearrange("g (p t) b -> g p t b", t=4)

    with tile.TileContext(nc) as tc:
        with (
            tc.tile_pool(name="const", bufs=1) as cpool,
            tc.tile_pool(name="work", bufs=1) as wpool,
            tc.tile_pool(name="outp", bufs=2) as opool,
            tc.tile_pool(name="psm", bufs=1, space="PSUM") as psm,
        ):
            coef_sb = cpool.tile([P, 8, P], f16)
            nc.sync.dma_start(out=coef_sb[:], in_=coef_in.ap())
            # per-(group, array) 512KB chunks in stream order: each
            # quadrant's matmuls start the moment its chunk lands, so
            # output drains interleave with the input stream
            dat = wpool.tile([P, 4 * n_groups, 2, gc], f16, name="dat")
            for s in range(4 * n_groups):
                nc.sync.dma_start(out=dat[:, s], in_=data_in.ap()[:, s])

            # PE-activity warmup: HBM bandwidth and engine clocks are capped
            # ~50% until a power-state promotion that follows a few us of
            # sustained tensor-engine activity (and decays again shortly
            # after it stops), so spin wide matmuls from the first possible
            # cycle until the first data chunk lands, and keep short spin
            # bursts between real matmuls later (dedicated PSUM bank, zero
            # data deps) to hold the promoted state through the write tail.
            wu_in = cpool.tile([P, 4 * P], f16)
            nc.gpsimd.memset(wu_in[:], 0.0)
            wu = psm.tile([P, HALF], f32, tag="SPIN", bufs=1)
            n_wu = 11
            for w in range(n_wu):
                nc.tensor.matmul(
                    wu[:], lhsT=wu_in[:, 0:P], rhs=wu_in[:, 0:HALF],
                    start=(w == 0), stop=(w == n_wu - 1),
                )

            def spin(n):
                for _ in range(n):
                    nc.tensor.matmul(
                        wu[:], lhsT=wu_in[:, 0:P], rhs=wu_in[:, 0:HALF],
                        start=True, stop=True,
                    )

            for g in range(n_groups):
                ortg = opool.tile([P, 4, gc], f16, tag="ortg")
                oitg = opool.tile([P, 4, gc], f16, tag="oitg")

                # quadrant-major order matches input arrival
                # (aue -> avo -> auo -> ave); the (E, O) slot pair is
                # adjacent DRAM rows drained together.
                last = g == n_groups - 1
                for qi in range(4):
                    stage, st_r = (
                        (ortg, ort_r) if qi % 2 == 0 else (oitg, oit_r)
                    )
                    base = 0 if qi < 2 else 2
                    # per-512-col-half PSUM tiles (1 bank each): the half's
                    # copy starts while the next half's matmul runs, and
                    # E/O × 2 bufs leave a free bank for the spin tile.
                    # h0 halves convert on vector, h1 on scalar.
                    for part in range(2):
                        tagp = "E" if part == 0 else "O"
                        for h in range(n_h):
                            bsl = slice(h * HALF, (h + 1) * HALF)
                            ph = psm.tile([P, HALF], f32, tag=tagp, bufs=2)
                            nc.tensor.matmul(
                                ph[:], lhsT=coef_sb[:, 2 * qi + part],
                                rhs=dat[:, 4 * g + qi, part, bsl],
                                start=True, stop=True,
                            )
                            if h == 0:
                                nc.vector.tensor_copy(
                                    out=stage[:, base + part, bsl], in_=ph[:]
                                )
                            else:
                                nc.scalar.copy(
                                    out=stage[:, base + part, bsl], in_=ph[:]
                                )
                    spin(2)
                    if last and qi >= 2:
                        # final group's last quadrants: drain each slot as
                        # soon as its copies land; the OI pair rides the
                        # (now idle) lower-latency sync queue
                        eng = nc.gpsimd if qi == 2 else nc.sync
                        eng.dma_start(
                            out=st_r[g][:, base : base + 1],
                            in_=stage[:, base : base + 1],
                        )
                        eng.dma_start(
                            out=st_r[g][:, base + 1 : base + 2],
                            in_=stage[:, base + 1 : base + 2],
                        )
                    else:
                        nc.gpsimd.dma_start(
                            out=st_r[g][:, base : base + 2],
                            in_=stage[:, base : base + 2],
                        )

            # tail spins: keep the PE (and the power state) hot while the
            # final copies convert and drain, so the power-state demotion
            # (~3us after the PE idles) lands past the start of the NEFF's
            # semaphore-clear epilogue and the clears run at full clock.
            # Sized to finish ~1us before the last drain lands (observed
            # margin ~4.7us) so exec time is never extended.
            spin(28)

    nc.compile()
    return nc


def _get_nc(b_core=B_CORE):
    if b_core not in _BUILD_CACHE:
        _BUILD_CACHE[b_core] = build_nc(b_core)
    return _BUILD_CACHE[b_core]


def _host_prep(x):
    """Two-level real-FFT folds (transposed) + host-side edge columns."""
    B = x.shape[0]
    U = np.empty((B, HALF), dtype=np.float32)
    V = np.empty((B, HALF), dtype=np.float32)
    U[:, 0] = x[:, 0]
    rev = x[:, 1023:HALF:-1]
    np.add(x[:, 1:HALF], rev, out=U[:, 1:HALF])
    np.subtract(x[:, 1:HALF], rev, out=V[:, 1:HALF])
    x512 = x[:, HALF]
    a = {k: np.empty((B, QU), dtype=np.float32)
         for k in ("aue", "auo", "avo", "ave")}
    a["aue"][:, 0] = U[:, 0] + x512
    a["auo"][:, 0] = U[:, 0] - x512
    a["avo"][:, 0] = 0.0                       # dead slot (EI E-row 0 is 0)
    a["ave"][:, 0] = V[:, QU]                  # rides OI E-row-0 pattern
    urev = U[:, 511:QU:-1]
    vrev = V[:, 511:QU:-1]
    np.add(U[:, 1:QU], urev, out=a["aue"][:, 1:QU])
    np.subtract(U[:, 1:QU], urev, out=a["auo"][:, 1:QU])
    np.subtract(V[:, 1:QU], vrev, out=a["avo"][:, 1:QU])
    np.add(V[:, 1:QU], vrev, out=a["ave"][:, 1:QU])
    col0 = (U.sum(axis=1, dtype=np.float64) + x512).astype(np.float32)
    u256 = U[:, QU].copy()                     # x[256] + x[768]
    # Xr[512] = sum x[even] - sum x[odd]
    xr512 = (x[:, 0::2].sum(axis=1, dtype=np.float64)
             - x[:, 1::2].sum(axis=1, dtype=np.float64)).astype(np.float32)
    at = {k: np.ascontiguousarray(v.T, dtype=np.float16)   # [256, B] fp16
          for k, v in a.items()}
    return at, col0, u256, xr512


def _blocked(a_t, sl, b_core):
    """[256, B] fp16 column-slice -> [128(p), n_groups, 2(j), gc]."""
    gc = min(GC, b_core)
    n_groups = b_core // gc
    s = a_t[:, sl].reshape(P, 2, n_groups, gc)          # [p, j, g, b]
    return np.ascontiguousarray(s.transpose(0, 2, 1, 3))  # [p, g, j, b]


def _assemble(half_t, out, sl, b_core, is_imag, u256=None, xr512=None):
    """Device half [n_groups, 512(r=4p+slot), gc] -> out[sl, :] (1024 cols).

    slots (0,1) = (E,O) of the even-k quadrant, (2,3) = (E,O) of odd-k.
    Even-k butterfly pairs c <-> 254-c (freq 2(c+1) <-> 2(255-c); col
    255 = freq 512 handled on host); odd-k pairs c <-> 255-c (freq
    2c+1 <-> 2(255-c)+1). Imag halves flip the sign at the mirrored
    column.
    """
    gc = min(GC, b_core)
    n_groups = b_core // gc
    h = half_t.reshape(n_groups, P, 4, gc)
    b0 = sl.start
    msign = -1.0 if is_imag else 1.0
    for g in range(n_groups):
        rows = slice(b0 + g * gc, b0 + (g + 1) * gc)
        blk = out[rows]
        # even-k quadrant: freqs 2,4,..,256 then partners 510,508,..,258
        e = h[g, :, 0, :].astype(np.float32)   # [128, gc]
        o = h[g, :, 1, :].astype(np.float32)
        blk[:, 2:258:2] = (e + o).T
        blk[:, 510:256:-2] = (msign * (e - o)[:127]).T
        # odd-k quadrant: freqs 1,3,..,255 then partners 511,509,..,257
        e = h[g, :, 2, :].astype(np.float32)
        o = h[g, :, 3, :].astype(np.float32)
        blk[:, 1:257:2] = (e + o).T
        blk[:, 511:255:-2] = (msign * (e - o)).T
        if is_imag:
            blk[:, 512] = 0.0
        else:
            # rank-1 U[256] correction on even freqs 2..510: sign
            # (-1)^(c+1) at freq 2(c+1) -> -1 at freq 4k+2, +1 at 4k
            u = u256[rows]
            blk[:, 2:512:4] -= u[:, None]
            blk[:, 4:512:4] += u[:, None]
            blk[:, 512] = xr512[rows]
    blk = out[sl]
    if is_imag:
        np.negative(blk[:, 511:0:-1], out=blk[:, 513:1024])
    else:
        blk[:, 513:1024] = blk[:, 511:0:-1]


def kernel(**inputs):
    from concourse.bass_utils import run_bass_kernel_spmd

    x = np.ascontiguousarray(np.asarray(inputs["x"], dtype=np.float32))
    assert x.shape == (BATCH, N_FFT), x.shape
    coef = _constants()
    at, col0, u256, xr512 = _host_prep(x)
    nc = _get_nc()
    gc = min(GC, B_CORE)
    n_groups = B_CORE // gc
    names = ("aue", "avo", "auo", "ave")
    in_maps = []
    for c in range(N_CORES):
        sl = slice(c * B_CORE, (c + 1) * B_CORE)
        bl = {k: _blocked(v, sl, B_CORE) for k, v in at.items()}
        # pack all data in stream order: slot s = 4g + array
        dd = np.stack(
            [bl[k][:, g] for g in range(n_groups) for k in names], axis=1
        )
        m = {"dd": np.ascontiguousarray(dd), "coef": coef}
        in_maps.append(m)
    trace = bool(int(os.environ.get("FFT_KERNEL_TRACE", "0")))

    def _run():
        return run_bass_kernel_spmd(
            nc, in_maps, core_ids=list(range(N_CORES)), trace=trace
        )

    def _corrupt(r):
        # outputs are bounded well inside fp16 range, so any non-finite
        # value means a transient device/DMA corruption -> retry
        return any(
            not np.isfinite(np.asarray(r.results[c][t], dtype=np.float32)).all()
            for c in range(N_CORES)
            for t in ("o_rt", "o_it")
        )

    try:
        res = _run()
        if _corrupt(res):
            res = _run()
    except Exception:
        # transient NRT/device hiccups have been observed; retry once
        res = _run()
    if trace:
        kernel.last_results = res
    real = np.empty((BATCH, N_FFT), dtype=np.float32)
    imag = np.empty((BATCH, N_FFT), dtype=np.float32)
    for c in range(N_CORES):
        sl = slice(c * B_CORE, (c + 1) * B_CORE)
        _assemble(res.results[c]["o_rt"], real, sl, B_CORE, is_imag=False,
                  u256=u256, xr512=xr512)
        _assemble(res.results[c]["o_it"], imag, sl, B_CORE, is_imag=True)
    real[:, 0] = col0
    imag[:, 0] = 0.0
    return real, imag


# revision 28
# speedup vs baseline: 1.1405x; 1.0028x over previous
"""Trainium2 Bass kernel: 1024-point FFT of real rows -> (real, imag).

Math: out = FFT_1024(x[b, :]) per row. Two folding levels over the real
input x (U[n] = x[n]+x[1024-n], V[n] = x[n]-x[1024-n]) give four real
arrays Aue/Auo/Avo/Ave of length 256 per row whose cos/sin transforms
are the four spectrum quadrants (even/odd k x real/imag):
  Xr[2m]   = Aue @ cos(2pi n m/512)   + U[256](-1)^m
  Xi[2m]   = Avo @ (-sin(2pi n m/512))
  Xr[2m+1] = Auo @ cos(pi n(2m+1)/512)
  Xi[2m+1] = Ave @ (-sin(pi n(2m+1)/512)) - V[256](-1)^m
A third radix-2 level halves the device matmul work: each quadrant
matrix C satisfies C[n, sigma(c)] = +/-(-1)^n C[n, c] for a column
involution sigma (sigma(c) = 254-c for even-k quadrants, 255-c for
odd-k), so splitting the data by row parity gives two [128x128]
products E = A[0::2] @ C[0::2, :128], O = A[1::2] @ C[1::2, :128]
and the host reconstructs both column halves with one butterfly:
quad[c] = E+O, quad[sigma(c)] = +/-(E-O). Edge terms (the U[256]
rank-1 term, k=0, k=512) are host-side rank-1 corrections; the
V[256](-1)^m term rides row 0 of the OI even matrix, which is
naturally zero. PE work per row: 8 x 128x128 MACs — half the naive
quadrant cost — so the tensor engine never paces the kernel.

All device I/O is fp16 (the kernel is HBM-bandwidth-bound with a
shared ~420 B/ns read+write cap; fp16 of folded data + coefficients
keeps L2 relative error ~4e-4). Data ships as one packed tensor in
stream order, DMA'd as per-(group, array) 512KB chunks (row n = 2p+j
is already the parity split) so each quadrant's matmuls start the
moment its chunk lands and output drains interleave with the input
stream. The 8 quarter-size matrices ship packed [128, 8, 128]
(2KB/partition, one 256KB DMA, first in the stream). Per group the
quadrants run in input-arrival order (ER, EI, OR, OI); each
sub-product is a single K=128 matmul into its own 512-col PSUM bank
(E/O tags x 2 bufs + 1 spin bank), h0 halves convert on vector and h1
on scalar, and each quadrant's (E, O) slot pair is adjacent DRAM rows
drained on the gpsimd queue as soon as its copies land (the final
group drains per-slot, the last quadrant on the idle sync queue, to
smooth the DMA tail). ~8.7MB/core total HBM traffic.

The chip throttles HBM bandwidth and engine clocks ~50% until a
power-state promotion that follows a few us of sustained
tensor-engine activity, and demotes again shortly after the PE goes
idle — so the PE spins wide zero matmuls from the first possible
cycle until real operands arrive, in short bursts between quadrants,
and through the final copy/drain tail (sized to end before the last
drain so exec time is never extended).

The host performs the pure data-expansion assembly: butterflies,
parity interleave, conjugate mirror, k=0/512 columns, rank-1 edge
corrections, final transpose, fp16->fp32 upcast. Pure data-parallel
across 8 cores, no collectives.
"""

import os
import numpy as np

N_FFT = 1024
BATCH = 16384
N_CORES = 8
B_CORE = BATCH // N_CORES  # 2048
P = 128
HALF = 512
QU = 256
GC = 1024                  # batch rows per group

_BUILD_CACHE = {}


def _constants():
    """8 packed [K=128, M=128] fp16 matrices: (E, O) per quadrant.

    packed[p, 2q+part, c] = Cq[2p+part, c] for c in [0, 128); row 0 of
    the OI even matrix (naturally zero) carries the -V[256](-1)^m term
    as the pattern (-1)^(c+1).
    """
    n = np.arange(QU, dtype=np.float64)[:, None]
    c = np.arange(QU, dtype=np.float64)[None, :]
    cer = np.cos(2 * np.pi * n * (c + 1) / 512)
    cei = -np.sin(2 * np.pi * n * (c + 1) / 512)
    cor = np.cos(np.pi * n * (2 * c + 1) / 512)
    coi = -np.sin(np.pi * n * (2 * c + 1) / 512)
    coi_e = coi[0::2, :P].copy()
    coi_e[0, :] = (-1.0) ** (np.arange(P) + 1)   # V[256] edge pattern
    mats = [
        cer[0::2, :P], cer[1::2, :P],   # ER: aue
        cei[0::2, :P], cei[1::2, :P],   # EI: avo (row 0 of E is zero)
        cor[0::2, :P], cor[1::2, :P],   # OR: auo
        coi_e, coi[1::2, :P],           # OI: ave
    ]
    packed = np.stack(mats, axis=1).astype(np.float16)  # [128, 8, 128]
    return np.ascontiguousarray(packed)


def build_nc(b_core=B_CORE):
    """Build + compile the per-core Bass program (same NEFF on all cores)."""
    import concourse.mybir as mybir
    import concourse.tile as tile
    from concourse import bacc

    f16 = mybir.dt.float16
    f32 = mybir.dt.float32

    gc = min(GC, b_core)
    n_groups = b_core // gc
    n_h = gc // HALF           # 512-col PSUM halves per group (2)

    nc = bacc.Bacc(
        "TRN2", target_bir_lowering=False, debug=False, num_devices=N_CORES
    )

    # all folded data packed in stream order: slot s = 4g + array,
    # arrays ordered (aue, avo, auo, ave) = quadrant processing order
    data_in = nc.dram_tensor(
        "dd", [P, 4 * n_groups, 2, gc], f16, kind="ExternalInput"
    )
    coef_in = nc.dram_tensor("coef", [P, 8, P], f16, kind="ExternalInput")
    # transposed halves, group-blocked: row r = 4p + slot;
    # rt slots: [ER_E, ER_O, OR_E, OR_O]; it slots: [EI_E, EI_O, OI_E, OI_O]
    o_rt = nc.dram_tensor("o_rt", [n_groups, 2 * QU, gc], f16, kind="ExternalOutput")
    o_it = nc.dram_tensor("o_it", [n_groups, 2 * QU, gc], f16, kind="ExternalOutput")

    ort_r = o_rt.ap().rearrange("g (p t) b -> g p t b", t=4)
    oit_r = o_it.ap().rearrange("g (p t) b -> g p t b", t=4)

    with tile.TileContext(nc) as tc:
        with (
            tc.tile_pool(name="const", bufs=1) as cpool,
            tc.tile_pool(name="work", bufs=1) as wpool,
            tc.tile_pool(name="outp", bufs=2) as opool,
            tc.tile_pool(name="psm", bufs=1, space="PSUM") as psm,
        ):
            coef_sb = cpool.tile([P, 8, P], f16)
            nc.sync.dma_start(out=coef_sb[:], in_=coef_in.ap())
            # per-(group, array) 512KB chunks in stream order: each
            # quadrant's matmuls start the moment its chunk lands, so
            # output drains interleave with the input stream
            dat = wpool.tile([P, 4 * n_groups, 2, gc], f16, name="dat")
            for s in range(4 * n_groups):
                nc.sync.dma_start(out=dat[:, s], in_=data_in.ap()[:, s])

            # PE-activity warmup: HBM bandwidth and engine clocks are capped
            # ~50% until a power-state promotion that follows a few us of
            # sustained tensor-engine activity (and decays again shortly
            # after it stops), so spin wide matmuls from the first possible
            # cycle until the first data chunk lands, and keep short spin
            # bursts between real matmuls later (dedicated PSUM bank, zero
            # data deps) to hold the promoted state through the write tail.
            wu_in = cpool.tile([P, 4 * P], f16)
            nc.gpsimd.memset(wu_in[:], 0.0)
            wu = psm.tile([P, HALF], f32, tag="SPIN", bufs=1)
            n_wu = 11
            for w in range(n_wu):
                nc.tensor.matmul(
                    wu[:], lhsT=wu_in[:, 0:P], rhs=wu_in[:, 0:HALF],
                    start=(w == 0), stop=(w == n_wu - 1),
                )

            def spin(n):
                for _ in range(n):
                    nc.tensor.matmul(
                        wu[:], lhsT=wu_in[:, 0:P], rhs=wu_in[:, 0:HALF],
                        start=True, stop=True,
                    )

            for g in range(n_groups):
                ortg = opool.tile([P, 4, gc], f16, tag="ortg")
                oitg = opool.tile([P, 4, gc], f16, tag="oitg")

                # quadrant-major order matches input arrival
                # (aue -> avo -> auo -> ave); the (E, O) slot pair is
                # adjacent DRAM rows drained together.
                last = g == n_groups - 1
                for qi in range(4):
                    stage, st_r = (
                        (ortg, ort_r) if qi % 2 == 0 else (oitg, oit_r)
                    )
                    base = 0 if qi < 2 else 2
                    # per-512-col-half PSUM tiles (1 bank each): the half's
                    # copy starts while the next half's matmul runs, and
                    # E/O × 2 bufs leave a free bank for the spin tile.
                    # h0 halves convert on vector, h1 on scalar.
                    for part in range(2):
                        tagp = "E" if part == 0 else "O"
                        for h in range(n_h):
                            bsl = slice(h * HALF, (h + 1) * HALF)
                            ph = psm.tile([P, HALF], f32, tag=tagp, bufs=2)
                            nc.tensor.matmul(
                                ph[:], lhsT=coef_sb[:, 2 * qi + part],
                                rhs=dat[:, 4 * g + qi, part, bsl],
                                start=True, stop=True,
                            )
                            if h == 0:
                                nc.vector.tensor_copy(
                                    out=stage[:, base + part, bsl], in_=ph[:]
                                )
                            else:
                                nc.scalar.copy(
                                    out=stage[:, base + part, bsl], in_=ph[:]
                                )
                    spin(2)
                    if last and qi >= 2:
                        # final group's last quadrants: drain each slot as
                        # soon as its copies land; the OI pair rides the
                        # (now idle) lower-latency sync queue
                        eng = nc.gpsimd if qi == 2 else nc.sync
                        eng.dma_start(
                            out=st_r[g][:, base : base + 1],
                            in_=stage[:, base : base + 1],
                        )
                        eng.dma_start(
                            out=st_r[g][:, base + 1 : base + 2],
                            in_=stage[:, base + 1 : base + 2],
                        )
                    else:
                        nc.gpsimd.dma_start(
                            out=st_r[g][:, base : base + 2],
                            in_=stage[:, base : base + 2],
                        )

            # tail spins: keep the PE (and the power state) hot while the
            # final copies convert and drain, so the power-state demotion
            # (~3us after the PE idles) lands past the start of the NEFF's
            # semaphore-clear epilogue and the clears run at full clock.
            # Sized to finish ~1.5us before the last drain lands (margin
            # measured 2.4-2.75us at spin(28) on both fast and slow draws)
            # so exec time is never extended.
            spin(32)

    nc.compile()
    return nc


def _get_nc(b_core=B_CORE):
    if b_core not in _BUILD_CACHE:
        _BUILD_CACHE[b_core] = build_nc(b_core)
    return _BUILD_CACHE[b_core]


def _host_prep(x):
    """Two-level real-FFT folds (transposed) + host-side edge columns."""
    B = x.shape[0]
    U = np.empty((B, HALF), dtype=np.float32)
    V = np.empty((B, HALF), dtype=np.float32)
    U[:, 0] = x[:, 0]
    rev = x[:, 1023:HALF:-1]
    np.add(x[:, 1:HALF], rev, out=U[:, 1:HALF])
    np.subtract(x[:, 1:HALF], rev, out=V[:, 1:HALF])
    x512 = x[:, HALF]
    a = {k: np.empty((B, QU), dtype=np.float32)
         for k in ("aue", "auo", "avo", "ave")}
    a["aue"][:, 0] = U[:, 0] + x512
    a["auo"][:, 0] = U[:, 0] - x512
    a["avo"][:, 0] = 0.0                       # dead slot (EI E-row 0 is 0)
    a["ave"][:, 0] = V[:, QU]                  # rides OI E-row-0 pattern
    urev = U[:, 511:QU:-1]
    vrev = V[:, 511:QU:-1]
    np.add(U[:, 1:QU], urev, out=a["aue"][:, 1:QU])
    np.subtract(U[:, 1:QU], urev, out=a["auo"][:, 1:QU])
    np.subtract(V[:, 1:QU], vrev, out=a["avo"][:, 1:QU])
    np.add(V[:, 1:QU], vrev, out=a["ave"][:, 1:QU])
    col0 = (U.sum(axis=1, dtype=np.float64) + x512).astype(np.float32)
    u256 = U[:, QU].copy()                     # x[256] + x[768]
    # Xr[512] = sum x[even] - sum x[odd]
    xr512 = (x[:, 0::2].sum(axis=1, dtype=np.float64)
             - x[:, 1::2].sum(axis=1, dtype=np.float64)).astype(np.float32)
    at = {k: np.ascontiguousarray(v.T, dtype=np.float16)   # [256, B] fp16
          for k, v in a.items()}
    return at, col0, u256, xr512


def _blocked(a_t, sl, b_core):
    """[256, B] fp16 column-slice -> [128(p), n_groups, 2(j), gc]."""
    gc = min(GC, b_core)
    n_groups = b_core // gc
    s = a_t[:, sl].reshape(P, 2, n_groups, gc)          # [p, j, g, b]
    return np.ascontiguousarray(s.transpose(0, 2, 1, 3))  # [p, g, j, b]


def _assemble(half_t, out, sl, b_core, is_imag, u256=None, xr512=None):
    """Device half [n_groups, 512(r=4p+slot), gc] -> out[sl, :] (1024 cols).

    slots (0,1) = (E,O) of the even-k quadrant, (2,3) = (E,O) of odd-k.
    Even-k butterfly pairs c <-> 254-c (freq 2(c+1) <-> 2(255-c); col
    255 = freq 512 handled on host); odd-k pairs c <-> 255-c (freq
    2c+1 <-> 2(255-c)+1). Imag halves flip the sign at the mirrored
    column.
    """
    gc = min(GC, b_core)
    n_groups = b_core // gc
    h = half_t.reshape(n_groups, P, 4, gc)
    b0 = sl.start
    msign = -1.0 if is_imag else 1.0
    for g in range(n_groups):
        rows = slice(b0 + g * gc, b0 + (g + 1) * gc)
        blk = out[rows]
        # even-k quadrant: freqs 2,4,..,256 then partners 510,508,..,258
        e = h[g, :, 0, :].astype(np.float32)   # [128, gc]
        o = h[g, :, 1, :].astype(np.float32)
        blk[:, 2:258:2] = (e + o).T
        blk[:, 510:256:-2] = (msign * (e - o)[:127]).T
        # odd-k quadrant: freqs 1,3,..,255 then partners 511,509,..,257
        e = h[g, :, 2, :].astype(np.float32)
        o = h[g, :, 3, :].astype(np.float32)
        blk[:, 1:257:2] = (e + o).T
        blk[:, 511:255:-2] = (msign * (e - o)).T
        if is_imag:
            blk[:, 512] = 0.0
        else:
            # rank-1 U[256] correction on even freqs 2..510: sign
            # (-1)^(c+1) at freq 2(c+1) -> -1 at freq 4k+2, +1 at 4k
            u = u256[rows]
            blk[:, 2:512:4] -= u[:, None]
            blk[:, 4:512:4] += u[:, None]
            blk[:, 512] = xr512[rows]
    blk = out[sl]
    if is_imag:
        np.negative(blk[:, 511:0:-1], out=blk[:, 513:1024])
    else:
        blk[:, 513:1024] = blk[:, 511:0:-1]


def kernel(**inputs):
    from concourse.bass_utils import run_bass_kernel_spmd

    x = np.ascontiguousarray(np.asarray(inputs["x"], dtype=np.float32))
    assert x.shape == (BATCH, N_FFT), x.shape
    coef = _constants()
    at, col0, u256, xr512 = _host_prep(x)
    nc = _get_nc()
    gc = min(GC, B_CORE)
    n_groups = B_CORE // gc
    names = ("aue", "avo", "auo", "ave")
    in_maps = []
    for c in range(N_CORES):
        sl = slice(c * B_CORE, (c + 1) * B_CORE)
        bl = {k: _blocked(v, sl, B_CORE) for k, v in at.items()}
        # pack all data in stream order: slot s = 4g + array
        dd = np.stack(
            [bl[k][:, g] for g in range(n_groups) for k in names], axis=1
        )
        m = {"dd": np.ascontiguousarray(dd), "coef": coef}
        in_maps.append(m)
    trace = bool(int(os.environ.get("FFT_KERNEL_TRACE", "0")))

    def _run():
        return run_bass_kernel_spmd(
            nc, in_maps, core_ids=list(range(N_CORES)), trace=trace
        )

    def _corrupt(r):
        # outputs are bounded well inside fp16 range, so any non-finite
        # value means a transient device/DMA corruption -> retry
        return any(
            not np.isfinite(np.asarray(r.results[c][t], dtype=np.float32)).all()
            for c in range(N_CORES)
            for t in ("o_rt", "o_it")
        )

    try:
        res = _run()
        if _corrupt(res):
            res = _run()
    except Exception:
        # transient NRT/device hiccups have been observed; retry once
        res = _run()
    if trace:
        kernel.last_results = res
    real = np.empty((BATCH, N_FFT), dtype=np.float32)
    imag = np.empty((BATCH, N_FFT), dtype=np.float32)
    for c in range(N_CORES):
        sl = slice(c * B_CORE, (c + 1) * B_CORE)
        _assemble(res.results[c]["o_rt"], real, sl, B_CORE, is_imag=False,
                  u256=u256, xr512=xr512)
        _assemble(res.results[c]["o_it"], imag, sl, B_CORE, is_imag=True)
    real[:, 0] = col0
    imag[:, 0] = 0.0
    return real, imag
